# revision 38
# baseline (speedup 1.0000x reference)
"""Trainium2 Bass kernel for nn_ActorCritic (moment-propagation actor-critic MLP).

Key observation: the reference returns (logits, value) = the *mu* outputs of the
final two rv_linear layers. mu propagation never reads Sigma, so the entire
covariance path is dead code for the outputs. The live computation is a plain
3-layer MLP:

    h1 = relu(x @ W1 + b1)        # (B, 512) @ (512, 256)
    h2 = relu(h1 @ W2 + b2)       # (B, 256) @ (256, 128)
    y  = h2 @ [Wa|Wc] + [ba|bc]   # (B, 128) @ (128, 19)
    logits = y[:, :18, None]; value = y[:, 18:, None]

Sharding: pure data parallel — batch 1024 split as 128 rows per core across 8
NeuronCores; weights replicated. Everything is computed feature-major
(features on SBUF partitions, batch on the free axis) so the TensorEngine
matmuls need no on-chip transposes; x is transposed host-side per shard.

Implementation notes:
  - Raw Bacc (no TileContext): explicit per-engine programs + semaphores.
    This avoids Tile's entry barrier and tail drain/EVSEM butterfly (~6us).
  - Matmul inputs are bf16 (f32 PSUM accumulation): halves DMA bytes and
    halves TensorE passes. Outputs are bias-dominated; bf16 keeps rel err
    ~1e-4, far inside the 2e-2 gate. Biases stay f32 via a separate tiny DMA.
  - Inputs are packed host-side into one [128, 1811] bf16 blob per core,
    organized as 4 per-K-chunk sections [xT_k | w1_k] + [w2 | w3], streamed
    as 5 DMAs alternating over the two HWDGE rings (sync + scalar) so
    layer-1 matmuls start as soon as chunk 0 lands.
"""

import sys

sys.path.insert(0, "/opt/trn_rl_repo")

from contextlib import ExitStack

import numpy as np
import ml_dtypes

import concourse.bacc as bacc
import concourse.mybir as mybir
from concourse.bass_utils import run_bass_kernel_spmd

N_CORES = 8
BATCH, IN_DIM, H1, H2, NACT = 1024, 512, 256, 128, 18
NOUT = NACT + 1  # logits columns + value column
BSH = BATCH // N_CORES  # 128 batch rows per core

_BF = mybir.dt.bfloat16
_F32 = mybir.dt.float32
_KC1 = IN_DIM // 128  # 4 contraction chunks for layer 1
_KC2 = H1 // 128      # 2 contraction chunks for layer 2

_CHW = BSH + H1                   # 384 cols per chunk section: [xT_k | w1_k]
_W20 = _KC1 * _CHW                # 1536
_W30 = _W20 + _KC2 * H2           # 1792
_BLOB_F = _W30 + NOUT             # 1811

_nc_cache = None


def _build_nc():
    """Raw-Bacc SPMD graph: explicit engine programs, bf16 matmuls."""
    nc = bacc.Bacc(enable_partition_id=False, monotonic_sem_count=0)

    blob = nc.declare_dram_parameter("blob", [128, _BLOB_F], _BF, isOutput=False)
    bias = nc.declare_dram_parameter("bias", [128, 4], _F32, isOutput=False)
    out = nc.declare_dram_parameter("out", [NOUT, BSH], _F32, isOutput=True)

    with ExitStack() as ctx:
        sbb = ctx.enter_context(nc.sbuf_tensor("sbb", [128, _BLOB_F], _BF))
        bias_t = ctx.enter_context(nc.sbuf_tensor("bias_t", [128, 4], _F32))
        h1_0 = ctx.enter_context(nc.sbuf_tensor("h1_0", [128, BSH], _BF))
        h1_1 = ctx.enter_context(nc.sbuf_tensor("h1_1", [128, BSH], _BF))
        h2_t = ctx.enter_context(nc.sbuf_tensor("h2_t", [128, BSH], _BF))
        out_t = ctx.enter_context(nc.sbuf_tensor("out_t", [NOUT, BSH], _F32))
        acc1_0 = ctx.enter_context(nc.psum_tensor("acc1_0", [128, BSH], _F32))
        acc1_1 = ctx.enter_context(nc.psum_tensor("acc1_1", [128, BSH], _F32))
        # separate banks per batch-half: PE may write one half while DVE
        # reads the other (same-bank PE-write + DVE-read is a HW hazard)
        HB = BSH // 2
        acc2_h = [
            ctx.enter_context(nc.psum_tensor(f"acc2_{h}", [128, HB], _F32))
            for h in range(2)
        ]
        acc3_h = [
            ctx.enter_context(nc.psum_tensor(f"acc3_{h}", [NOUT, HB], _F32))
            for h in range(2)
        ]
        sA = ctx.enter_context(nc.semaphore("sA"))       # sync-ring DMA completions
        sB = ctx.enter_context(nc.semaphore("sB"))       # scalar-ring DMA completions
        sC = ctx.enter_context(nc.semaphore("sC"))       # swdge DMA completions
        pe_sem = ctx.enter_context(nc.semaphore("pe_sem"))
        act_sem = ctx.enter_context(nc.semaphore("act_sem"))
        block = ctx.enter_context(nc.Block(no_gpsimd_drain=True))

        h1_j = (h1_0, h1_1)

        def xT_sl(k):
            return sbb[:, k * _CHW : k * _CHW + BSH]

        def w1_sl(k, j):
            base = k * _CHW + BSH + j * 128
            return sbb[:, base : base + 128]

        def w2_sl(j):
            return sbb[:, _W20 + j * H2 : _W20 + (j + 1) * H2]

        # DMA plan (three paths in parallel):
        #   ring A (sync):    chunk0 (small first DMA gates PE start),
        #                     later the out store (single_packet)
        #   ring B (scalar):  chunks 2+3 + w2 + w3 (one DMA)
        #   SWDGE (gpsimd):   chunk1, then bias (f32, tiny)
        # Per-ring FIFO + per-slot +1 sem increments make threshold waits safe.
        _k_sem = {0: (sA, 16), 1: (sA, 16), 2: (sB, 16), 3: (sB, 16)}

        def ring_wait(pe, k):
            sem, val = _k_sem[k]
            pe.wait_ge(sem, val)

        # Semaphore protocol:
        #   pe_sem:  1,2 = acc1_0/acc1_1 done; 3,4 = acc2 half a/b done;
        #            5,6 = acc3 half a/b done
        #   act_sem: 1,2 = relu1 j0/j1 done; 3,4 = h2 half a/b done;
        #            5,6 = out_t half a/b done
        def hb(h):
            return slice(h * HB, (h + 1) * HB)

        @block.sync
        def _(sync):
            sync.dma_start(
                out=sbb[:, 0 : 2 * _CHW], in_=blob[:, 0 : 2 * _CHW]
            ).then_inc(sA, 16)
            sync.wait_ge(act_sem, 5)
            # No completion wait: the Block-exit InstDrain on SP flushes the
            # HWDGE queue (incl. this store) before the NEFF can end.
            sync.dma_start(
                out=out[:, hb(0)], in_=out_t[:, hb(0)], single_packet=True
            ).then_inc(sA, 16)

        @block.gpsimd
        def _(gpsimd):
            gpsimd.dma_start(
                out=bias_t[:, :], in_=bias[:, :], single_packet=True
            ).then_inc(sC, 16)

        @block.scalar
        def _(scalar):
            scalar.dma_start(
                out=sbb[:, 2 * _CHW : _BLOB_F], in_=blob[:, 2 * _CHW : _BLOB_F]
            ).then_inc(sB, 16)
            scalar.wait_ge(act_sem, 6)
            scalar.dma_start(
                out=out[:, hb(1)], in_=out_t[:, hb(1)], single_packet=True
            ).then_inc(sB, 16)

        @block.vector
        def _(vector):
            # all activations on DVE: relu(acc + bias) fused via tensor_scalar
            vector.wait_ge(sC, 16)  # bias loaded
            for j in range(_KC2):
                vector.wait_ge(pe_sem, j + 1)
                vector.tensor_scalar(
                    h1_j[j][:, :],
                    (acc1_0, acc1_1)[j][:, :],
                    bias_t[:, j : j + 1],
                    0.0,
                    mybir.AluOpType.add,
                    mybir.AluOpType.max,
                ).then_inc(act_sem, 1)
            for h in range(2):
                vector.wait_ge(pe_sem, 3 + h)
                vector.tensor_scalar(
                    h2_t[:, hb(h)],
                    acc2_h[h][:, :],
                    bias_t[:, 2:3],
                    0.0,
                    mybir.AluOpType.add,
                    mybir.AluOpType.max,
                ).then_inc(act_sem, 1)
            for h in range(2):
                vector.wait_ge(pe_sem, 5 + h)
                vector.tensor_scalar_add(
                    out_t[:, hb(h)],
                    acc3_h[h][:, :],
                    bias_t[0:NOUT, 3:4],
                ).then_inc(act_sem, 1)

        @block.tensor
        def _(pe):
            # layer 1, k-major so both PSUM groups finish right after chunk 3
            for k in range(_KC1):
                ring_wait(pe, k)
                for j in range(_KC2):
                    mm = pe.matmul(
                        (acc1_0, acc1_1)[j][:, :],
                        w1_sl(k, j),
                        xT_sl(k),
                        start=(k == 0),
                        stop=(k == _KC1 - 1),
                    )
                    if k == _KC1 - 1:
                        mm.then_inc(pe_sem, 1)
            # layer 2, pipelined over batch halves
            for h in range(2):
                for j in range(_KC2):
                    if h == 0:
                        pe.wait_ge(act_sem, j + 1)
                    mm = pe.matmul(
                        acc2_h[h][:, :],
                        w2_sl(j),
                        h1_j[j][:, hb(h)],
                        start=(j == 0),
                        stop=(j == _KC2 - 1),
                    )
                mm.then_inc(pe_sem, 1)
            # layer 3, pipelined over batch halves
            for h in range(2):
                pe.wait_ge(act_sem, 3 + h)
                pe.matmul(
                    acc3_h[h][:, :],
                    sbb[:, _W30 : _W30 + NOUT],
                    h2_t[:, hb(h)],
                    start=True,
                    stop=True,
                ).then_inc(pe_sem, 1)

    nc.finalize()
    return nc


def _get_nc():
    global _nc_cache
    if _nc_cache is None:
        _nc_cache = _build_nc()
    return _nc_cache


def _prep_in_maps(x, w_mu1, b_mu1, w_mu2, b_mu2, w_mua, b_mua, w_muc, b_muc):
    bf16 = ml_dtypes.bfloat16
    x = np.asarray(x, dtype=np.float32)
    w1 = np.asarray(w_mu1, dtype=np.float32)
    b1 = np.asarray(b_mu1, dtype=np.float32).reshape(H1)
    w2 = np.asarray(w_mu2, dtype=np.float32)
    b2 = np.asarray(b_mu2, dtype=np.float32).reshape(H2)
    w3 = np.concatenate(
        [np.asarray(w_mua, np.float32), np.asarray(w_muc, np.float32)], axis=1
    )  # (128, 19)
    b3 = np.concatenate(
        [np.asarray(b_mua, np.float32).reshape(NACT),
         np.asarray(b_muc, np.float32).reshape(1)]
    )  # (19,)

    bias = np.zeros((128, 4), np.float32)
    bias[:, 0] = b1[:128]
    bias[:, 1] = b1[128:]
    bias[:, 2] = b2
    bias[:NOUT, 3] = b3

    # Shared (weight) section of the blob, identical on every core.
    shared = np.zeros((128, _BLOB_F), bf16)  # chunk xT columns filled per core
    for k in range(_KC1):
        shared[:, k * _CHW + BSH : (k + 1) * _CHW] = w1[k * 128 : (k + 1) * 128, :]
    for j in range(_KC2):
        shared[:, _W20 + j * H2 : _W20 + (j + 1) * H2] = w2[j * 128 : (j + 1) * 128, :]
    shared[:, _W30:_BLOB_F] = w3

    xs = x[:, :, 0]  # (1024, 512)
    in_maps = []
    for c in range(N_CORES):
        blob = shared.copy()
        xsh = xs[c * BSH : (c + 1) * BSH, :]  # (128 batch, 512 feat)
        xT = xsh.T.astype(bf16)  # (512 feat, 128 batch)
        for k in range(_KC1):
            blob[:, k * _CHW : k * _CHW + BSH] = xT[k * 128 : (k + 1) * 128, :]
        in_maps.append({"blob": blob, "bias": bias})
    return in_maps


def _postprocess(results):
    yT = np.concatenate([results[c]["out"] for c in range(N_CORES)], axis=1)  # (19, 1024)
    y = yT.T.astype(np.float32)  # (1024, 19)
    logits = np.ascontiguousarray(y[:, :NACT])[:, :, None]
    value = np.ascontiguousarray(y[:, NACT:])[:, :, None]
    return logits, value


def kernel(x, w_mu1, w_sigma1, b_mu1, b_sigma1,
           w_mu2, w_sigma2, b_mu2, b_sigma2,
           w_mua, w_sigmaa, b_mua, b_sigmaa,
           w_muc, w_sigmac, b_muc, b_sigmac):
    in_maps = _prep_in_maps(x, w_mu1, b_mu1, w_mu2, b_mu2, w_mua, b_mua, w_muc, b_muc)
    nc = _get_nc()
    results = run_bass_kernel_spmd(nc, in_maps, core_ids=list(range(N_CORES))).results
    return _postprocess(results)


# revision 40
# speedup vs baseline: 1.0033x; 1.0033x over previous
"""Trainium2 Bass kernel for nn_ActorCritic (moment-propagation actor-critic MLP).

Key observation: the reference returns (logits, value) = the *mu* outputs of the
final two rv_linear layers. mu propagation never reads Sigma, so the entire
covariance path is dead code for the outputs. The live computation is a plain
3-layer MLP:

    h1 = relu(x @ W1 + b1)        # (B, 512) @ (512, 256)
    h2 = relu(h1 @ W2 + b2)       # (B, 256) @ (256, 128)
    y  = h2 @ [Wa|Wc] + [ba|bc]   # (B, 128) @ (128, 19)
    logits = y[:, :18, None]; value = y[:, 18:, None]

Sharding: pure data parallel — batch 1024 split as 128 rows per core across 8
NeuronCores; weights replicated. Everything is computed feature-major
(features on SBUF partitions, batch on the free axis) so the TensorEngine
matmuls need no on-chip transposes; x is transposed host-side per shard.

Implementation notes:
  - Raw Bacc (no TileContext): explicit per-engine programs + semaphores.
    This avoids Tile's entry barrier and tail drain/EVSEM butterfly (~6us).
  - Matmul inputs are bf16 (f32 PSUM accumulation): halves DMA bytes and
    halves TensorE passes. Outputs are bias-dominated; bf16 keeps rel err
    ~1e-4, far inside the 2e-2 gate. Biases stay f32 via a separate tiny DMA.
  - Inputs are packed host-side into one [128, 1811] bf16 blob per core,
    organized as 4 per-K-chunk sections [xT_k | w1_k] + [w2 | w3], streamed
    as 5 DMAs alternating over the two HWDGE rings (sync + scalar) so
    layer-1 matmuls start as soon as chunk 0 lands.
"""

import sys

sys.path.insert(0, "/opt/trn_rl_repo")

from contextlib import ExitStack

import numpy as np
import ml_dtypes

import concourse.bacc as bacc
import concourse.mybir as mybir
from concourse.bass_utils import run_bass_kernel_spmd

N_CORES = 8
BATCH, IN_DIM, H1, H2, NACT = 1024, 512, 256, 128, 18
NOUT = NACT + 1  # logits columns + value column
BSH = BATCH // N_CORES  # 128 batch rows per core

_BF = mybir.dt.bfloat16
_F32 = mybir.dt.float32
_KC1 = IN_DIM // 128  # 4 contraction chunks for layer 1
_KC2 = H1 // 128      # 2 contraction chunks for layer 2

_CHW = BSH + H1                   # 384 cols per chunk section: [xT_k | w1_k]
_W20 = _KC1 * _CHW                # 1536
_W30 = _W20 + _KC2 * H2           # 1792
_BLOB_F = _W30 + NOUT             # 1811

_nc_cache = None


def _build_nc():
    """Raw-Bacc SPMD graph: explicit engine programs, bf16 matmuls."""
    nc = bacc.Bacc(enable_partition_id=False, monotonic_sem_count=0)

    blob = nc.declare_dram_parameter("blob", [128, _BLOB_F], _BF, isOutput=False)
    bias = nc.declare_dram_parameter("bias", [128, 4], _F32, isOutput=False)
    out = nc.declare_dram_parameter("out", [NOUT, BSH], _F32, isOutput=True)

    with ExitStack() as ctx:
        sbb = ctx.enter_context(nc.sbuf_tensor("sbb", [128, _BLOB_F], _BF))
        bias_t = ctx.enter_context(nc.sbuf_tensor("bias_t", [128, 4], _F32))
        h1_0 = ctx.enter_context(nc.sbuf_tensor("h1_0", [128, BSH], _BF))
        h1_1 = ctx.enter_context(nc.sbuf_tensor("h1_1", [128, BSH], _BF))
        h2_t = ctx.enter_context(nc.sbuf_tensor("h2_t", [128, BSH], _BF))
        out_t = ctx.enter_context(nc.sbuf_tensor("out_t", [NOUT, BSH], _F32))
        acc1_0 = ctx.enter_context(nc.psum_tensor("acc1_0", [128, BSH], _F32))
        acc1_1 = ctx.enter_context(nc.psum_tensor("acc1_1", [128, BSH], _F32))
        acc2_h = [ctx.enter_context(nc.psum_tensor("acc2", [128, BSH], _F32))]
        acc3_h = [ctx.enter_context(nc.psum_tensor("acc3", [NOUT, BSH], _F32))]
        sA = ctx.enter_context(nc.semaphore("sA"))       # sync-ring DMA completions
        sB = ctx.enter_context(nc.semaphore("sB"))       # scalar-ring DMA completions
        sC = ctx.enter_context(nc.semaphore("sC"))       # swdge DMA completions
        pe_sem = ctx.enter_context(nc.semaphore("pe_sem"))
        act_sem = ctx.enter_context(nc.semaphore("act_sem"))
        block = ctx.enter_context(nc.Block(no_gpsimd_drain=True))

        h1_j = (h1_0, h1_1)

        def xT_sl(k):
            return sbb[:, k * _CHW : k * _CHW + BSH]

        def w1_sl(k, j):
            base = k * _CHW + BSH + j * 128
            return sbb[:, base : base + 128]

        def w2_sl(j):
            return sbb[:, _W20 + j * H2 : _W20 + (j + 1) * H2]

        # DMA plan (three paths in parallel):
        #   ring A (sync):    chunk0 (small first DMA gates PE start),
        #                     later the out store (single_packet)
        #   ring B (scalar):  chunks 2+3 + w2 + w3 (one DMA)
        #   SWDGE (gpsimd):   chunk1, then bias (f32, tiny)
        # Per-ring FIFO + per-slot +1 sem increments make threshold waits safe.
        _k_sem = {0: (sA, 16), 1: (sA, 16), 2: (sB, 16), 3: (sB, 16)}

        def ring_wait(pe, k):
            sem, val = _k_sem[k]
            pe.wait_ge(sem, val)

        @block.sync
        def _(sync):
            sync.dma_start(
                out=sbb[:, 0 : 2 * _CHW], in_=blob[:, 0 : 2 * _CHW]
            ).then_inc(sA, 16)
            sync.wait_ge(act_sem, 4)
            # No completion wait: the Block-exit InstDrain on SP flushes the
            # HWDGE queue (incl. this store) before the NEFF can end.
            sync.dma_start(
                out=out[:, :], in_=out_t[:, :], single_packet=True
            ).then_inc(sA, 16)

        @block.gpsimd
        def _(gpsimd):
            gpsimd.dma_start(
                out=bias_t[:, :], in_=bias[:, :], single_packet=True
            ).then_inc(sC, 16)

        @block.scalar
        def _(scalar):
            scalar.dma_start(
                out=sbb[:, 2 * _CHW : _BLOB_F], in_=blob[:, 2 * _CHW : _BLOB_F]
            ).then_inc(sB, 16)

        @block.vector
        def _(vector):
            # all activations on DVE: relu(acc + bias) fused via tensor_scalar
            vector.wait_ge(sC, 16)  # bias loaded
            for j in range(_KC2):
                vector.wait_ge(pe_sem, j + 1)
                vector.tensor_scalar(
                    h1_j[j][:, :],
                    (acc1_0, acc1_1)[j][:, :],
                    bias_t[:, j : j + 1],
                    0.0,
                    mybir.AluOpType.add,
                    mybir.AluOpType.max,
                ).then_inc(act_sem, 1)
            vector.wait_ge(pe_sem, 3)
            vector.tensor_scalar(
                h2_t[:, :],
                acc2_h[0][:, :],
                bias_t[:, 2:3],
                0.0,
                mybir.AluOpType.add,
                mybir.AluOpType.max,
            ).then_inc(act_sem, 1)
            vector.wait_ge(pe_sem, 4)
            vector.tensor_scalar_add(
                out_t[:, :],
                acc3_h[0][:, :],
                bias_t[0:NOUT, 3:4],
            ).then_inc(act_sem, 1)

        @block.tensor
        def _(pe):
            # layer 1, k-major so both PSUM groups finish right after chunk 3
            for k in range(_KC1):
                ring_wait(pe, k)
                for j in range(_KC2):
                    mm = pe.matmul(
                        (acc1_0, acc1_1)[j][:, :],
                        w1_sl(k, j),
                        xT_sl(k),
                        start=(k == 0),
                        stop=(k == _KC1 - 1),
                    )
                    if k == _KC1 - 1:
                        mm.then_inc(pe_sem, 1)
            # layer 2
            for j in range(_KC2):
                pe.wait_ge(act_sem, j + 1)
                mm = pe.matmul(
                    acc2_h[0][:, :],
                    w2_sl(j),
                    h1_j[j][:, :],
                    start=(j == 0),
                    stop=(j == _KC2 - 1),
                )
            mm.then_inc(pe_sem, 1)
            # layer 3
            pe.wait_ge(act_sem, 3)
            pe.matmul(
                acc3_h[0][:, :],
                sbb[:, _W30 : _W30 + NOUT],
                h2_t[:, :],
                start=True,
                stop=True,
            ).then_inc(pe_sem, 1)

    nc.finalize()
    return nc


def _get_nc():
    global _nc_cache
    if _nc_cache is None:
        _nc_cache = _build_nc()
    return _nc_cache


def _prep_in_maps(x, w_mu1, b_mu1, w_mu2, b_mu2, w_mua, b_mua, w_muc, b_muc):
    bf16 = ml_dtypes.bfloat16
    x = np.asarray(x, dtype=np.float32)
    w1 = np.asarray(w_mu1, dtype=np.float32)
    b1 = np.asarray(b_mu1, dtype=np.float32).reshape(H1)
    w2 = np.asarray(w_mu2, dtype=np.float32)
    b2 = np.asarray(b_mu2, dtype=np.float32).reshape(H2)
    w3 = np.concatenate(
        [np.asarray(w_mua, np.float32), np.asarray(w_muc, np.float32)], axis=1
    )  # (128, 19)
    b3 = np.concatenate(
        [np.asarray(b_mua, np.float32).reshape(NACT),
         np.asarray(b_muc, np.float32).reshape(1)]
    )  # (19,)

    bias = np.zeros((128, 4), np.float32)
    bias[:, 0] = b1[:128]
    bias[:, 1] = b1[128:]
    bias[:, 2] = b2
    bias[:NOUT, 3] = b3

    # Shared (weight) section of the blob, identical on every core.
    shared = np.zeros((128, _BLOB_F), bf16)  # chunk xT columns filled per core
    for k in range(_KC1):
        shared[:, k * _CHW + BSH : (k + 1) * _CHW] = w1[k * 128 : (k + 1) * 128, :]
    for j in range(_KC2):
        shared[:, _W20 + j * H2 : _W20 + (j + 1) * H2] = w2[j * 128 : (j + 1) * 128, :]
    shared[:, _W30:_BLOB_F] = w3

    xs = x[:, :, 0]  # (1024, 512)
    in_maps = []
    for c in range(N_CORES):
        blob = shared.copy()
        xsh = xs[c * BSH : (c + 1) * BSH, :]  # (128 batch, 512 feat)
        xT = xsh.T.astype(bf16)  # (512 feat, 128 batch)
        for k in range(_KC1):
            blob[:, k * _CHW : k * _CHW + BSH] = xT[k * 128 : (k + 1) * 128, :]
        in_maps.append({"blob": blob, "bias": bias})
    return in_maps


def _postprocess(results):
    yT = np.concatenate([results[c]["out"] for c in range(N_CORES)], axis=1)  # (19, 1024)
    y = yT.T.astype(np.float32)  # (1024, 19)
    logits = np.ascontiguousarray(y[:, :NACT])[:, :, None]
    value = np.ascontiguousarray(y[:, NACT:])[:, :, None]
    return logits, value


def kernel(x, w_mu1, w_sigma1, b_mu1, b_sigma1,
           w_mu2, w_sigma2, b_mu2, b_sigma2,
           w_mua, w_sigmaa, b_mua, b_sigmaa,
           w_muc, w_sigmac, b_muc, b_sigmac):
    in_maps = _prep_in_maps(x, w_mu1, b_mu1, w_mu2, b_mu2, w_mua, b_mua, w_muc, b_muc)
    nc = _get_nc()
    results = run_bass_kernel_spmd(nc, in_maps, core_ids=list(range(N_CORES))).results
    return _postprocess(results)


# revision 42
# speedup vs baseline: 1.0497x; 1.0463x over previous
"""Trainium2 Bass kernel for nn_ActorCritic (moment-propagation actor-critic MLP).

Key observation: the reference returns (logits, value) = the *mu* outputs of the
final two rv_linear layers. mu propagation never reads Sigma, so the entire
covariance path is dead code for the outputs. The live computation is a plain
3-layer MLP:

    h1 = relu(x @ W1 + b1)        # (B, 512) @ (512, 256)
    h2 = relu(h1 @ W2 + b2)       # (B, 256) @ (256, 128)
    y  = h2 @ [Wa|Wc] + [ba|bc]   # (B, 128) @ (128, 19)
    logits = y[:, :18, None]; value = y[:, 18:, None]

Sharding: pure data parallel — batch 1024 split as 128 rows per core across 8
NeuronCores; weights replicated. Everything is computed feature-major
(features on SBUF partitions, batch on the free axis) so the TensorEngine
matmuls need no on-chip transposes; x is transposed host-side per shard.

Implementation notes:
  - Raw Bacc (no TileContext): explicit per-engine programs + semaphores.
    This avoids Tile's entry barrier and tail drain/EVSEM butterfly (~6us).
  - Matmul inputs are bf16 (f32 PSUM accumulation): halves DMA bytes and
    halves TensorE passes. Outputs are bias-dominated; bf16 keeps rel err
    ~1e-4, far inside the 2e-2 gate. Biases stay f32 via a separate tiny DMA.
  - Inputs are packed host-side into one [128, 1811] bf16 blob per core,
    organized as 4 per-K-chunk sections [xT_k | w1_k] + [w2 | w3], streamed
    as 5 DMAs alternating over the two HWDGE rings (sync + scalar) so
    layer-1 matmuls start as soon as chunk 0 lands.
"""

import sys

sys.path.insert(0, "/opt/trn_rl_repo")

from contextlib import ExitStack

import numpy as np
import ml_dtypes

import concourse.bacc as bacc
import concourse.mybir as mybir
from concourse.bass_utils import run_bass_kernel_spmd

N_CORES = 8
BATCH, IN_DIM, H1, H2, NACT = 1024, 512, 256, 128, 18
NOUT = NACT + 1  # logits columns + value column
BSH = BATCH // N_CORES  # 128 batch rows per core

_BF = mybir.dt.bfloat16
_F32 = mybir.dt.float32
_KC1 = IN_DIM // 128  # 4 contraction chunks for layer 1
_KC2 = H1 // 128      # 2 contraction chunks for layer 2

_CHW = BSH + H1                   # 384 cols per chunk section: [xT_k | w1_k]
_W20 = _KC1 * _CHW                # 1536
_W30 = _W20 + _KC2 * H2           # 1792
_BLOB_F = _W30 + NOUT             # 1811

_nc_cache = None


def _build_nc():
    """Raw-Bacc SPMD graph: explicit engine programs, bf16 matmuls."""
    nc = bacc.Bacc(enable_partition_id=False, monotonic_sem_count=0)

    blob = nc.declare_dram_parameter("blob", [128, _BLOB_F], _BF, isOutput=False)
    bias = nc.declare_dram_parameter("bias", [128, 4], _F32, isOutput=False)
    out = nc.declare_dram_parameter("out", [NOUT, BSH], _F32, isOutput=True)

    with ExitStack() as ctx:
        sbb = ctx.enter_context(nc.sbuf_tensor("sbb", [128, _BLOB_F], _BF))
        bias_t = ctx.enter_context(nc.sbuf_tensor("bias_t", [128, 4], _F32))
        h1_0 = ctx.enter_context(nc.sbuf_tensor("h1_0", [128, BSH], _BF))
        h1_1 = ctx.enter_context(nc.sbuf_tensor("h1_1", [128, BSH], _BF))
        h2_t = ctx.enter_context(nc.sbuf_tensor("h2_t", [128, BSH], _BF))
        out_t = ctx.enter_context(nc.sbuf_tensor("out_t", [NOUT, BSH], _F32))
        acc1_0 = ctx.enter_context(nc.psum_tensor("acc1_0", [128, BSH], _F32))
        acc1_1 = ctx.enter_context(nc.psum_tensor("acc1_1", [128, BSH], _F32))
        acc2_h = [ctx.enter_context(nc.psum_tensor("acc2", [128, BSH], _F32))]
        acc3_h = [ctx.enter_context(nc.psum_tensor("acc3", [NOUT, BSH], _F32))]
        sA = ctx.enter_context(nc.semaphore("sA"))       # sync-ring DMA completions
        sB = ctx.enter_context(nc.semaphore("sB"))       # scalar-ring DMA completions
        sC = ctx.enter_context(nc.semaphore("sC"))       # swdge DMA completions
        pe_sem = ctx.enter_context(nc.semaphore("pe_sem"))
        act_sem = ctx.enter_context(nc.semaphore("act_sem"))
        block = ctx.enter_context(nc.Block(no_gpsimd_drain=True))

        h1_j = (h1_0, h1_1)

        def xT_sl(k):
            return sbb[:, k * _CHW : k * _CHW + BSH]

        def w1_sl(k, j):
            base = k * _CHW + BSH + j * 128
            return sbb[:, base : base + 128]

        def w2_sl(j):
            return sbb[:, _W20 + j * H2 : _W20 + (j + 1) * H2]

        # DMA plan (three paths in parallel):
        #   ring A (sync):    chunk0 (small first DMA gates PE start),
        #                     later the out store (single_packet)
        #   ring B (scalar):  chunks 2+3 + w2 + w3 (one DMA)
        #   SWDGE (gpsimd):   chunk1, then bias (f32, tiny)
        # Per-ring FIFO + per-slot +1 sem increments make threshold waits safe.
        _k_sem = {0: (sA, 16), 1: (sA, 16), 2: (sB, 16), 3: (sB, 16)}

        def ring_wait(pe, k):
            sem, val = _k_sem[k]
            pe.wait_ge(sem, val)

        hoist = []  # input-DMA triggers to relocate into the entry bb

        @block.sync
        def _(sync):
            hoist.append(
                sync.dma_start(
                    out=sbb[:, 0 : 2 * _CHW], in_=blob[:, 0 : 2 * _CHW]
                ).then_inc(sA, 16)
            )
            sync.wait_ge(act_sem, 4)
            # No completion wait: the Block-exit InstDrain on SP flushes the
            # HWDGE queue (incl. this store) before the NEFF can end.
            sync.dma_start(
                out=out[:, :], in_=out_t[:, :], single_packet=True
            ).then_inc(sA, 16)

        @block.gpsimd
        def _(gpsimd):
            hoist.append(
                gpsimd.dma_start(
                    out=bias_t[:, :], in_=bias[:, :], single_packet=True
                ).then_inc(sC, 16)
            )

        @block.scalar
        def _(scalar):
            hoist.append(
                scalar.dma_start(
                    out=sbb[:, 2 * _CHW : _BLOB_F], in_=blob[:, 2 * _CHW : _BLOB_F]
                ).then_inc(sB, 16)
            )

        @block.vector
        def _(vector):
            # all activations on DVE: relu(acc + bias) fused via tensor_scalar
            vector.wait_ge(sC, 16)  # bias loaded
            for j in range(_KC2):
                vector.wait_ge(pe_sem, j + 1)
                vector.tensor_scalar(
                    h1_j[j][:, :],
                    (acc1_0, acc1_1)[j][:, :],
                    bias_t[:, j : j + 1],
                    0.0,
                    mybir.AluOpType.add,
                    mybir.AluOpType.max,
                ).then_inc(act_sem, 1)
            vector.wait_ge(pe_sem, 3)
            vector.tensor_scalar(
                h2_t[:, :],
                acc2_h[0][:, :],
                bias_t[:, 2:3],
                0.0,
                mybir.AluOpType.add,
                mybir.AluOpType.max,
            ).then_inc(act_sem, 1)
            vector.wait_ge(pe_sem, 4)
            vector.tensor_scalar_add(
                out_t[:, :],
                acc3_h[0][:, :],
                bias_t[0:NOUT, 3:4],
            ).then_inc(act_sem, 1)

        @block.tensor
        def _(pe):
            # layer 1, k-major so both PSUM groups finish right after chunk 3
            for k in range(_KC1):
                ring_wait(pe, k)
                for j in range(_KC2):
                    mm = pe.matmul(
                        (acc1_0, acc1_1)[j][:, :],
                        w1_sl(k, j),
                        xT_sl(k),
                        start=(k == 0),
                        stop=(k == _KC1 - 1),
                    )
                    if k == _KC1 - 1:
                        mm.then_inc(pe_sem, 1)
            # layer 2
            for j in range(_KC2):
                pe.wait_ge(act_sem, j + 1)
                mm = pe.matmul(
                    acc2_h[0][:, :],
                    w2_sl(j),
                    h1_j[j][:, :],
                    start=(j == 0),
                    stop=(j == _KC2 - 1),
                )
            mm.then_inc(pe_sem, 1)
            # layer 3
            pe.wait_ge(act_sem, 3)
            pe.matmul(
                acc3_h[0][:, :],
                sbb[:, _W30 : _W30 + NOUT],
                h2_t[:, :],
                start=True,
                stop=True,
            ).then_inc(pe_sem, 1)

    # Hoist the input-DMA triggers into the entry bb, right after the engine
    # preamble call and BEFORE the const-pool barrier: the loads start ~1.5us
    # earlier and overlap the rest of the framework preamble. They only
    # depend on the semaphore range-clear, which is inside the preamble call.
    f = nc.m.functions[0]
    main_bb = list(f.blocks)[0]
    for h in hoist:
        inst = h.ins
        moved = False
        for b in f.blocks:
            il = b.instructions
            for i, x in enumerate(il):
                if x is inst:
                    il.pop(i)
                    moved = True
                    break
            if moved:
                break
        assert moved, f"could not find {inst.name} to hoist"
        main_bb.instructions.insert(1, inst)

    nc.finalize()
    return nc


def _get_nc():
    global _nc_cache
    if _nc_cache is None:
        _nc_cache = _build_nc()
    return _nc_cache


def _prep_in_maps(x, w_mu1, b_mu1, w_mu2, b_mu2, w_mua, b_mua, w_muc, b_muc):
    bf16 = ml_dtypes.bfloat16
    x = np.asarray(x, dtype=np.float32)
    w1 = np.asarray(w_mu1, dtype=np.float32)
    b1 = np.asarray(b_mu1, dtype=np.float32).reshape(H1)
    w2 = np.asarray(w_mu2, dtype=np.float32)
    b2 = np.asarray(b_mu2, dtype=np.float32).reshape(H2)
    w3 = np.concatenate(
        [np.asarray(w_mua, np.float32), np.asarray(w_muc, np.float32)], axis=1
    )  # (128, 19)
    b3 = np.concatenate(
        [np.asarray(b_mua, np.float32).reshape(NACT),
         np.asarray(b_muc, np.float32).reshape(1)]
    )  # (19,)

    bias = np.zeros((128, 4), np.float32)
    bias[:, 0] = b1[:128]
    bias[:, 1] = b1[128:]
    bias[:, 2] = b2
    bias[:NOUT, 3] = b3

    # Shared (weight) section of the blob, identical on every core.
    shared = np.zeros((128, _BLOB_F), bf16)  # chunk xT columns filled per core
    for k in range(_KC1):
        shared[:, k * _CHW + BSH : (k + 1) * _CHW] = w1[k * 128 : (k + 1) * 128, :]
    for j in range(_KC2):
        shared[:, _W20 + j * H2 : _W20 + (j + 1) * H2] = w2[j * 128 : (j + 1) * 128, :]
    shared[:, _W30:_BLOB_F] = w3

    xs = x[:, :, 0]  # (1024, 512)
    in_maps = []
    for c in range(N_CORES):
        blob = shared.copy()
        xsh = xs[c * BSH : (c + 1) * BSH, :]  # (128 batch, 512 feat)
        xT = xsh.T.astype(bf16)  # (512 feat, 128 batch)
        for k in range(_KC1):
            blob[:, k * _CHW : k * _CHW + BSH] = xT[k * 128 : (k + 1) * 128, :]
        in_maps.append({"blob": blob, "bias": bias})
    return in_maps


def _postprocess(results):
    yT = np.concatenate([results[c]["out"] for c in range(N_CORES)], axis=1)  # (19, 1024)
    y = yT.T.astype(np.float32)  # (1024, 19)
    logits = np.ascontiguousarray(y[:, :NACT])[:, :, None]
    value = np.ascontiguousarray(y[:, NACT:])[:, :, None]
    return logits, value


def kernel(x, w_mu1, w_sigma1, b_mu1, b_sigma1,
           w_mu2, w_sigma2, b_mu2, b_sigma2,
           w_mua, w_sigmaa, b_mua, b_sigmaa,
           w_muc, w_sigmac, b_muc, b_sigmac):
    in_maps = _prep_in_maps(x, w_mu1, b_mu1, w_mu2, b_mu2, w_mua, b_mua, w_muc, b_muc)
    nc = _get_nc()
    results = run_bass_kernel_spmd(nc, in_maps, core_ids=list(range(N_CORES))).results
    return _postprocess(results)


# revision 44
# speedup vs baseline: 1.2053x; 1.1482x over previous
"""Trainium2 Bass kernel for nn_ActorCritic (moment-propagation actor-critic MLP).

Key observation: the reference returns (logits, value) = the *mu* outputs of the
final two rv_linear layers. mu propagation never reads Sigma, so the entire
covariance path is dead code for the outputs. The live computation is a plain
3-layer MLP:

    h1 = relu(x @ W1 + b1)        # (B, 512) @ (512, 256)
    h2 = relu(h1 @ W2 + b2)       # (B, 256) @ (256, 128)
    y  = h2 @ [Wa|Wc] + [ba|bc]   # (B, 128) @ (128, 19)
    logits = y[:, :18, None]; value = y[:, 18:, None]

Sharding: pure data parallel — batch 1024 split as 128 rows per core across 8
NeuronCores; weights replicated. Everything is computed feature-major
(features on SBUF partitions, batch on the free axis) so the TensorEngine
matmuls need no on-chip transposes; x is transposed host-side per shard.

Implementation notes:
  - Raw Bacc (no TileContext): explicit per-engine programs + semaphores.
    This avoids Tile's entry barrier and tail drain/EVSEM butterfly (~6us).
  - Matmul inputs are bf16 (f32 PSUM accumulation): halves DMA bytes and
    halves TensorE passes. Outputs are bias-dominated; bf16 keeps rel err
    ~1e-4, far inside the 2e-2 gate. Biases stay f32 via a separate tiny DMA.
  - Inputs are packed host-side into one [128, 1811] bf16 blob per core,
    organized as 4 per-K-chunk sections [xT_k | w1_k] + [w2 | w3], streamed
    as 5 DMAs alternating over the two HWDGE rings (sync + scalar) so
    layer-1 matmuls start as soon as chunk 0 lands.
"""

import sys

sys.path.insert(0, "/opt/trn_rl_repo")

from contextlib import ExitStack

import numpy as np
import ml_dtypes

import concourse.bacc as bacc
import concourse.mybir as mybir
from concourse.bass_utils import run_bass_kernel_spmd

N_CORES = 8
BATCH, IN_DIM, H1, H2, NACT = 1024, 512, 256, 128, 18
NOUT = NACT + 1  # logits columns + value column
BSH = BATCH // N_CORES  # 128 batch rows per core

_BF = mybir.dt.bfloat16
_F32 = mybir.dt.float32
_KC1 = IN_DIM // 128  # 4 contraction chunks for layer 1
_KC2 = H1 // 128      # 2 contraction chunks for layer 2

_CHW = BSH + H1                   # 384 cols per chunk section: [xT_k | w1_k]
_W20 = _KC1 * _CHW                # 1536
_W30 = _W20 + _KC2 * H2           # 1792
_BLOB_F = _W30 + NOUT             # 1811

_nc_cache = None


def _build_nc():
    """Raw-Bacc SPMD graph: explicit engine programs, bf16 matmuls."""
    nc = bacc.Bacc(enable_partition_id=False, monotonic_sem_count=0)

    blob = nc.declare_dram_parameter("blob", [128, _BLOB_F], _BF, isOutput=False)
    bias = nc.declare_dram_parameter("bias", [128, 4], _F32, isOutput=False)
    out = nc.declare_dram_parameter("out", [NOUT, BSH], _F32, isOutput=True)

    with ExitStack() as ctx:
        sbb = ctx.enter_context(nc.sbuf_tensor("sbb", [128, _BLOB_F], _BF))
        bias_t = ctx.enter_context(nc.sbuf_tensor("bias_t", [128, 4], _F32))
        h1_0 = ctx.enter_context(nc.sbuf_tensor("h1_0", [128, BSH], _BF))
        h1_1 = ctx.enter_context(nc.sbuf_tensor("h1_1", [128, BSH], _BF))
        h2_t = ctx.enter_context(nc.sbuf_tensor("h2_t", [128, BSH], _BF))
        out_t = ctx.enter_context(nc.sbuf_tensor("out_t", [NOUT, BSH], _F32))
        acc1_0 = ctx.enter_context(nc.psum_tensor("acc1_0", [128, BSH], _F32))
        acc1_1 = ctx.enter_context(nc.psum_tensor("acc1_1", [128, BSH], _F32))
        acc2_h = [ctx.enter_context(nc.psum_tensor("acc2", [128, BSH], _F32))]
        acc3_h = [ctx.enter_context(nc.psum_tensor("acc3", [NOUT, BSH], _F32))]
        sA = ctx.enter_context(nc.semaphore("sA"))       # sync-ring DMA completions
        sB = ctx.enter_context(nc.semaphore("sB"))       # scalar-ring DMA completions
        sC = ctx.enter_context(nc.semaphore("sC"))       # swdge DMA completions
        pe_sem = ctx.enter_context(nc.semaphore("pe_sem"))
        act_sem = ctx.enter_context(nc.semaphore("act_sem"))
        block = ctx.enter_context(nc.Block(no_gpsimd_drain=True))

        h1_j = (h1_0, h1_1)

        def xT_sl(k):
            return sbb[:, k * _CHW : k * _CHW + BSH]

        def w1_sl(k, j):
            base = k * _CHW + BSH + j * 128
            return sbb[:, base : base + 128]

        def w2_sl(j):
            return sbb[:, _W20 + j * H2 : _W20 + (j + 1) * H2]

        # DMA plan (three paths in parallel):
        #   ring A (sync):    chunk0 (small first DMA gates PE start),
        #                     later the out store (single_packet)
        #   ring B (scalar):  chunks 2+3 + w2 + w3 (one DMA)
        #   SWDGE (gpsimd):   chunk1, then bias (f32, tiny)
        # Per-ring FIFO + per-slot +1 sem increments make threshold waits safe.
        _k_sem = {0: (sA, 16), 1: (sA, 16), 2: (sB, 16), 3: (sB, 16)}

        def ring_wait(pe, k):
            sem, val = _k_sem[k]
            pe.wait_ge(sem, val)

        hoist = []  # input-DMA triggers to relocate into the entry bb

        @block.sync
        def _(sync):
            hoist.append(
                sync.dma_start(
                    out=sbb[:, 0 : 2 * _CHW], in_=blob[:, 0 : 2 * _CHW]
                ).then_inc(sA, 16)
            )
            sync.wait_ge(act_sem, 4)
            # No completion wait: the Block-exit InstDrain on SP flushes the
            # HWDGE queue (incl. this store) before the NEFF can end.
            sync.dma_start(
                out=out[:, :], in_=out_t[:, :], single_packet=True
            ).then_inc(sA, 16)

        @block.gpsimd
        def _(gpsimd):
            hoist.append(
                gpsimd.dma_start(
                    out=bias_t[:, :], in_=bias[:, :], single_packet=True
                ).then_inc(sC, 16)
            )
            hoist.append(
                gpsimd.dma_start(
                    out=sbb[:, 4 * _CHW : _BLOB_F], in_=blob[:, 4 * _CHW : _BLOB_F]
                ).then_inc(sC, 16)
            )

        @block.scalar
        def _(scalar):
            hoist.append(
                scalar.dma_start(
                    out=sbb[:, 2 * _CHW : 4 * _CHW], in_=blob[:, 2 * _CHW : 4 * _CHW]
                ).then_inc(sB, 16)
            )

        @block.vector
        def _(vector):
            # all activations on DVE: relu(acc + bias) fused via tensor_scalar
            vector.wait_ge(sC, 16)  # bias loaded
            for j in range(_KC2):
                vector.wait_ge(pe_sem, j + 1)
                vector.tensor_scalar(
                    h1_j[j][:, :],
                    (acc1_0, acc1_1)[j][:, :],
                    bias_t[:, j : j + 1],
                    0.0,
                    mybir.AluOpType.add,
                    mybir.AluOpType.max,
                ).then_inc(act_sem, 1)
            vector.wait_ge(pe_sem, 3)
            vector.tensor_scalar(
                h2_t[:, :],
                acc2_h[0][:, :],
                bias_t[:, 2:3],
                0.0,
                mybir.AluOpType.add,
                mybir.AluOpType.max,
            ).then_inc(act_sem, 1)
            vector.wait_ge(pe_sem, 4)
            vector.tensor_scalar_add(
                out_t[:, :],
                acc3_h[0][:, :],
                bias_t[0:NOUT, 3:4],
            ).then_inc(act_sem, 1)

        @block.tensor
        def _(pe):
            # All input data is resident when the preamble barrier releases
            # (the hoisted DMAs complete inside it), so run layer 1 j-major:
            # relu(j0) on DVE overlaps the j1 matmuls.
            for j in range(_KC2):
                for k in range(_KC1):
                    if j == 0:
                        ring_wait(pe, k)
                    mm = pe.matmul(
                        (acc1_0, acc1_1)[j][:, :],
                        w1_sl(k, j),
                        xT_sl(k),
                        start=(k == 0),
                        stop=(k == _KC1 - 1),
                    )
                mm.then_inc(pe_sem, 1)
            # layer 2
            pe.wait_ge(sC, 32)  # w2/w3 loaded
            for j in range(_KC2):
                pe.wait_ge(act_sem, j + 1)
                mm = pe.matmul(
                    acc2_h[0][:, :],
                    w2_sl(j),
                    h1_j[j][:, :],
                    start=(j == 0),
                    stop=(j == _KC2 - 1),
                )
            mm.then_inc(pe_sem, 1)
            # layer 3
            pe.wait_ge(act_sem, 3)
            pe.matmul(
                acc3_h[0][:, :],
                sbb[:, _W30 : _W30 + NOUT],
                h2_t[:, :],
                start=True,
                stop=True,
            ).then_inc(pe_sem, 1)

    # Hoist the input-DMA triggers into the entry bb, right after the engine
    # preamble call and BEFORE the const-pool barrier: the loads start ~1.5us
    # earlier and overlap the rest of the framework preamble. They only
    # depend on the semaphore range-clear, which is inside the preamble call.
    f = nc.m.functions[0]
    main_bb = list(f.blocks)[0]
    for h in hoist:
        inst = h.ins
        moved = False
        for b in f.blocks:
            il = b.instructions
            for i, x in enumerate(il):
                if x is inst:
                    il.pop(i)
                    moved = True
                    break
            if moved:
                break
        assert moved, f"could not find {inst.name} to hoist"
        main_bb.instructions.insert(1, inst)

    nc.finalize()
    return nc


def _get_nc():
    global _nc_cache
    if _nc_cache is None:
        _nc_cache = _build_nc()
    return _nc_cache


def _prep_in_maps(x, w_mu1, b_mu1, w_mu2, b_mu2, w_mua, b_mua, w_muc, b_muc):
    bf16 = ml_dtypes.bfloat16
    x = np.asarray(x, dtype=np.float32)
    w1 = np.asarray(w_mu1, dtype=np.float32)
    b1 = np.asarray(b_mu1, dtype=np.float32).reshape(H1)
    w2 = np.asarray(w_mu2, dtype=np.float32)
    b2 = np.asarray(b_mu2, dtype=np.float32).reshape(H2)
    w3 = np.concatenate(
        [np.asarray(w_mua, np.float32), np.asarray(w_muc, np.float32)], axis=1
    )  # (128, 19)
    b3 = np.concatenate(
        [np.asarray(b_mua, np.float32).reshape(NACT),
         np.asarray(b_muc, np.float32).reshape(1)]
    )  # (19,)

    bias = np.zeros((128, 4), np.float32)
    bias[:, 0] = b1[:128]
    bias[:, 1] = b1[128:]
    bias[:, 2] = b2
    bias[:NOUT, 3] = b3

    # Shared (weight) section of the blob, identical on every core.
    shared = np.zeros((128, _BLOB_F), bf16)  # chunk xT columns filled per core
    for k in range(_KC1):
        shared[:, k * _CHW + BSH : (k + 1) * _CHW] = w1[k * 128 : (k + 1) * 128, :]
    for j in range(_KC2):
        shared[:, _W20 + j * H2 : _W20 + (j + 1) * H2] = w2[j * 128 : (j + 1) * 128, :]
    shared[:, _W30:_BLOB_F] = w3

    xs = x[:, :, 0]  # (1024, 512)
    in_maps = []
    for c in range(N_CORES):
        blob = shared.copy()
        xsh = xs[c * BSH : (c + 1) * BSH, :]  # (128 batch, 512 feat)
        xT = xsh.T.astype(bf16)  # (512 feat, 128 batch)
        for k in range(_KC1):
            blob[:, k * _CHW : k * _CHW + BSH] = xT[k * 128 : (k + 1) * 128, :]
        in_maps.append({"blob": blob, "bias": bias})
    return in_maps


def _postprocess(results):
    yT = np.concatenate([results[c]["out"] for c in range(N_CORES)], axis=1)  # (19, 1024)
    y = yT.T.astype(np.float32)  # (1024, 19)
    logits = np.ascontiguousarray(y[:, :NACT])[:, :, None]
    value = np.ascontiguousarray(y[:, NACT:])[:, :, None]
    return logits, value


def kernel(x, w_mu1, w_sigma1, b_mu1, b_sigma1,
           w_mu2, w_sigma2, b_mu2, b_sigma2,
           w_mua, w_sigmaa, b_mua, b_sigmaa,
           w_muc, w_sigmac, b_muc, b_sigmac):
    in_maps = _prep_in_maps(x, w_mu1, b_mu1, w_mu2, b_mu2, w_mua, b_mua, w_muc, b_muc)
    nc = _get_nc()
    results = run_bass_kernel_spmd(nc, in_maps, core_ids=list(range(N_CORES))).results
    return _postprocess(results)


# revision 48
# speedup vs baseline: 1.2608x; 1.0460x over previous
"""Trainium2 Bass kernel for nn_ActorCritic (moment-propagation actor-critic MLP).

Key observation: the reference returns (logits, value) = the *mu* outputs of the
final two rv_linear layers. mu propagation never reads Sigma, so the entire
covariance path is dead code for the outputs. The live computation is a plain
3-layer MLP:

    h1 = relu(x @ W1 + b1)        # (B, 512) @ (512, 256)
    h2 = relu(h1 @ W2 + b2)       # (B, 256) @ (256, 128)
    y  = h2 @ [Wa|Wc] + [ba|bc]   # (B, 128) @ (128, 19)
    logits = y[:, :18, None]; value = y[:, 18:, None]

Sharding: pure data parallel — batch 1024 split as 128 rows per core across 8
NeuronCores; weights replicated. Everything is computed feature-major
(features on SBUF partitions, batch on the free axis) so the TensorEngine
matmuls need no on-chip transposes; x is transposed host-side per shard.

Implementation notes:
  - Raw Bacc (no TileContext): explicit per-engine programs + semaphores.
    This avoids Tile's entry barrier and tail drain/EVSEM butterfly (~6us).
  - Matmul inputs are bf16 (f32 PSUM accumulation): halves DMA bytes and
    halves TensorE passes. Outputs are bias-dominated; bf16 keeps rel err
    ~1e-4, far inside the 2e-2 gate. Biases stay f32 via a separate tiny DMA.
  - Inputs are packed host-side into one [128, 1811] bf16 blob per core,
    organized as 4 per-K-chunk sections [xT_k | w1_k] + [w2 | w3], streamed
    as 5 DMAs alternating over the two HWDGE rings (sync + scalar) so
    layer-1 matmuls start as soon as chunk 0 lands.
"""

import sys

sys.path.insert(0, "/opt/trn_rl_repo")

from contextlib import ExitStack

import numpy as np
import ml_dtypes

import concourse.bacc as bacc
import concourse.mybir as mybir
from concourse.bass_utils import run_bass_kernel_spmd

N_CORES = 8
BATCH, IN_DIM, H1, H2, NACT = 1024, 512, 256, 128, 18
NOUT = NACT + 1  # logits columns + value column
BSH = BATCH // N_CORES  # 128 batch rows per core

_BF = mybir.dt.bfloat16
_F32 = mybir.dt.float32
_KC1 = IN_DIM // 128  # 4 contraction chunks for layer 1
_KC2 = H1 // 128      # 2 contraction chunks for layer 2

_CHW = BSH + H1                   # 384 cols per chunk section: [xT_k | w1_k]
_W20 = _KC1 * _CHW                # 1536
_W30 = _W20 + _KC2 * H2           # 1792
_BLOB_F = _W30 + NOUT             # 1811

_nc_cache = None


def _build_nc():
    """Raw-Bacc SPMD graph: explicit engine programs, bf16 matmuls."""
    nc = bacc.Bacc(enable_partition_id=False, monotonic_sem_count=0)

    blob = nc.declare_dram_parameter("blob", [128, _BLOB_F], _BF, isOutput=False)
    bias = nc.declare_dram_parameter("bias", [128, 4], _F32, isOutput=False)
    out = nc.declare_dram_parameter("out", [NOUT, BSH], _F32, isOutput=True)

    with ExitStack() as ctx:
        sbb = ctx.enter_context(nc.sbuf_tensor("sbb", [128, _BLOB_F], _BF))
        bias_t = ctx.enter_context(nc.sbuf_tensor("bias_t", [128, 4], _F32))
        h1_0 = ctx.enter_context(nc.sbuf_tensor("h1_0", [128, BSH], _BF))
        h1_1 = ctx.enter_context(nc.sbuf_tensor("h1_1", [128, BSH], _BF))
        h2_t = ctx.enter_context(nc.sbuf_tensor("h2_t", [128, BSH], _BF))
        out_t = ctx.enter_context(nc.sbuf_tensor("out_t", [NOUT, BSH], _F32))
        acc1_0 = ctx.enter_context(nc.psum_tensor("acc1_0", [128, BSH], _F32))
        acc1_1 = ctx.enter_context(nc.psum_tensor("acc1_1", [128, BSH], _F32))
        acc2_h = [ctx.enter_context(nc.psum_tensor("acc2", [128, BSH], _F32))]
        acc3_h = [ctx.enter_context(nc.psum_tensor("acc3", [NOUT, BSH], _F32))]
        sA = ctx.enter_context(nc.semaphore("sA"))       # sync-ring DMA completions
        sB = ctx.enter_context(nc.semaphore("sB"))       # scalar-ring DMA completions
        sC = ctx.enter_context(nc.semaphore("sC"))       # swdge DMA completions
        pe_sem = ctx.enter_context(nc.semaphore("pe_sem"))
        act_sem = ctx.enter_context(nc.semaphore("act_sem"))
        block = ctx.enter_context(nc.Block(no_gpsimd_drain=True))

        h1_j = (h1_0, h1_1)

        def xT_sl(k):
            return sbb[:, k * _CHW : k * _CHW + BSH]

        def w1_sl(k, j):
            base = k * _CHW + BSH + j * 128
            return sbb[:, base : base + 128]

        def w2_sl(j):
            return sbb[:, _W20 + j * H2 : _W20 + (j + 1) * H2]

        # DMA plan (three paths in parallel):
        #   ring A (sync):    chunk0 (small first DMA gates PE start),
        #                     later the out store (single_packet)
        #   ring B (scalar):  chunks 2+3 + w2 + w3 (one DMA)
        #   SWDGE (gpsimd):   chunk1, then bias (f32, tiny)
        # Per-ring FIFO + per-slot +1 sem increments make threshold waits safe.
        _k_sem = {0: (sA, 16), 1: (sA, 16), 2: (sB, 16), 3: (sB, 16)}

        def ring_wait(pe, k):
            sem, val = _k_sem[k]
            pe.wait_ge(sem, val)

        hoist = []  # input-DMA triggers to relocate into the entry bb

        @block.sync
        def _(sync):
            hoist.append(
                sync.dma_start(
                    out=sbb[:, 0 : 2 * _CHW], in_=blob[:, 0 : 2 * _CHW]
                ).then_inc(sA, 16)
            )
            hoist.append(
                sync.dma_start(
                    out=bias_t[:, :], in_=bias[:, :], single_packet=True
                ).then_inc(sA, 16)
            )
            sync.wait_ge(act_sem, 4)
            # No completion wait: the Block-exit InstDrain on SP flushes the
            # HWDGE queue (incl. this store) before the NEFF can end.
            sync.dma_start(
                out=out[:, :], in_=out_t[:, :], single_packet=True
            ).then_inc(sA, 16)

        @block.gpsimd
        def _(gpsimd):
            hoist.append(
                gpsimd.dma_start(
                    out=sbb[:, 4 * _CHW : _BLOB_F], in_=blob[:, 4 * _CHW : _BLOB_F]
                ).then_inc(sC, 16)
            )

        @block.scalar
        def _(scalar):
            hoist.append(
                scalar.dma_start(
                    out=sbb[:, 2 * _CHW : 4 * _CHW], in_=blob[:, 2 * _CHW : 4 * _CHW]
                ).then_inc(sB, 16)
            )

        @block.vector
        def _(vector):
            # all activations on DVE: relu(acc + bias) fused via tensor_scalar
            vector.wait_ge(sA, 32)  # bias loaded (2nd ring-A DMA)
            for j in range(_KC2):
                vector.wait_ge(pe_sem, j + 1)
                vector.tensor_scalar(
                    h1_j[j][:, :],
                    (acc1_0, acc1_1)[j][:, :],
                    bias_t[:, j : j + 1],
                    0.0,
                    mybir.AluOpType.add,
                    mybir.AluOpType.max,
                ).then_inc(act_sem, 1)
            vector.wait_ge(pe_sem, 3)
            vector.tensor_scalar(
                h2_t[:, :],
                acc2_h[0][:, :],
                bias_t[:, 2:3],
                0.0,
                mybir.AluOpType.add,
                mybir.AluOpType.max,
            ).then_inc(act_sem, 1)
            vector.wait_ge(pe_sem, 4)
            vector.tensor_scalar_add(
                out_t[:, :],
                acc3_h[0][:, :],
                bias_t[0:NOUT, 3:4],
            ).then_inc(act_sem, 1)

        @block.tensor
        def _(pe):
            # All input data is resident when the preamble barrier releases
            # (the hoisted DMAs complete inside it), so run layer 1 j-major:
            # relu(j0) on DVE overlaps the j1 matmuls.
            for j in range(_KC2):
                for k in range(_KC1):
                    if j == 0:
                        ring_wait(pe, k)
                    mm = pe.matmul(
                        (acc1_0, acc1_1)[j][:, :],
                        w1_sl(k, j),
                        xT_sl(k),
                        start=(k == 0),
                        stop=(k == _KC1 - 1),
                    )
                mm.then_inc(pe_sem, 1)
            # layer 2
            pe.wait_ge(sC, 16)  # w2/w3 loaded
            for j in range(_KC2):
                pe.wait_ge(act_sem, j + 1)
                mm = pe.matmul(
                    acc2_h[0][:, :],
                    w2_sl(j),
                    h1_j[j][:, :],
                    start=(j == 0),
                    stop=(j == _KC2 - 1),
                )
            mm.then_inc(pe_sem, 1)
            # layer 3
            pe.wait_ge(act_sem, 3)
            pe.matmul(
                acc3_h[0][:, :],
                sbb[:, _W30 : _W30 + NOUT],
                h2_t[:, :],
                start=True,
                stop=True,
            ).then_inc(pe_sem, 1)

    # Hoist the input-DMA triggers into the entry bb, right after the engine
    # preamble call and BEFORE the const-pool barrier: the loads start ~1.5us
    # earlier and overlap the rest of the framework preamble. They only
    # depend on the semaphore range-clear, which is inside the preamble call.
    f = nc.m.functions[0]
    main_bb = list(f.blocks)[0]
    for h in hoist:
        inst = h.ins
        moved = False
        for b in f.blocks:
            il = b.instructions
            for i, x in enumerate(il):
                if x is inst:
                    il.pop(i)
                    moved = True
                    break
            if moved:
                break
        assert moved, f"could not find {inst.name} to hoist"
        main_bb.instructions.insert(1, inst)

    nc.finalize()
    return nc


def _get_nc():
    global _nc_cache
    if _nc_cache is None:
        _nc_cache = _build_nc()
    return _nc_cache


def _prep_in_maps(x, w_mu1, b_mu1, w_mu2, b_mu2, w_mua, b_mua, w_muc, b_muc):
    bf16 = ml_dtypes.bfloat16
    x = np.asarray(x, dtype=np.float32)
    w1 = np.asarray(w_mu1, dtype=np.float32)
    b1 = np.asarray(b_mu1, dtype=np.float32).reshape(H1)
    w2 = np.asarray(w_mu2, dtype=np.float32)
    b2 = np.asarray(b_mu2, dtype=np.float32).reshape(H2)
    w3 = np.concatenate(
        [np.asarray(w_mua, np.float32), np.asarray(w_muc, np.float32)], axis=1
    )  # (128, 19)
    b3 = np.concatenate(
        [np.asarray(b_mua, np.float32).reshape(NACT),
         np.asarray(b_muc, np.float32).reshape(1)]
    )  # (19,)

    bias = np.zeros((128, 4), np.float32)
    bias[:, 0] = b1[:128]
    bias[:, 1] = b1[128:]
    bias[:, 2] = b2
    bias[:NOUT, 3] = b3

    # Shared (weight) section of the blob, identical on every core.
    shared = np.zeros((128, _BLOB_F), bf16)  # chunk xT columns filled per core
    for k in range(_KC1):
        shared[:, k * _CHW + BSH : (k + 1) * _CHW] = w1[k * 128 : (k + 1) * 128, :]
    for j in range(_KC2):
        shared[:, _W20 + j * H2 : _W20 + (j + 1) * H2] = w2[j * 128 : (j + 1) * 128, :]
    shared[:, _W30:_BLOB_F] = w3

    xs = x[:, :, 0]  # (1024, 512)
    in_maps = []
    for c in range(N_CORES):
        blob = shared.copy()
        xsh = xs[c * BSH : (c + 1) * BSH, :]  # (128 batch, 512 feat)
        xT = xsh.T.astype(bf16)  # (512 feat, 128 batch)
        for k in range(_KC1):
            blob[:, k * _CHW : k * _CHW + BSH] = xT[k * 128 : (k + 1) * 128, :]
        in_maps.append({"blob": blob, "bias": bias})
    return in_maps


def _postprocess(results):
    yT = np.concatenate([results[c]["out"] for c in range(N_CORES)], axis=1)  # (19, 1024)
    y = yT.T.astype(np.float32)  # (1024, 19)
    logits = np.ascontiguousarray(y[:, :NACT])[:, :, None]
    value = np.ascontiguousarray(y[:, NACT:])[:, :, None]
    return logits, value


def kernel(x, w_mu1, w_sigma1, b_mu1, b_sigma1,
           w_mu2, w_sigma2, b_mu2, b_sigma2,
           w_mua, w_sigmaa, b_mua, b_sigmaa,
           w_muc, w_sigmac, b_muc, b_sigmac):
    in_maps = _prep_in_maps(x, w_mu1, b_mu1, w_mu2, b_mu2, w_mua, b_mua, w_muc, b_muc)
    nc = _get_nc()
    results = run_bass_kernel_spmd(nc, in_maps, core_ids=list(range(N_CORES))).results
    return _postprocess(results)
